# revision 2
# baseline (speedup 1.0000x reference)
"""NeRF renderer on 8 TRN2 NeuronCores (Bass/Tile kernel).

Data-parallel over rays: rays_o/rays_d split into 8 shards of 2048 rays;
a host-precomputed dilated-occupancy volume (uint8, one lookup per sample
replaces the 8-corner trilinear test exactly for binary/nonnegative grids)
and the packed MLP weights are replicated per core. Per-core Bass kernel:
DVE geometry in ray-major tiles -> indirect-DMA occupancy gather ->
2-sample-packed PE matmuls (sigma head folded into layer 2, biases folded)
-> transmittance scan -> weighted RGB reduction. Host gathers the 8 shard
outputs. Falls back to a pure-numpy renderer on any device-path failure.
"""
import os
import sys
import threading
import numpy as np

N_RAYS = 16384
NS = 128
GS = 128
NEAR = 1.0e-1
EARLY_TERM = 1.0e-4
N_CORES = 8
NRAYS_CORE = N_RAYS // N_CORES
MPAD = 130

_BASS_PATHS = ["/opt/trn_rl_repo", "/root/.axon_site/_ro/trn_rl_repo"]

_cache = {}


# --------------------------------------------------------------------------
# host-side prep (t tables, dilated occupancy volume, packed weights)
# --------------------------------------------------------------------------

def _t_tables(n_samples):
    half = int(n_samples) // 2
    t_close = np.linspace(NEAR, NEAR + 1.0, half, dtype=np.float32)
    t_far = np.exp(
        np.arange(half, dtype=np.float32) * np.float32(np.log(1.0 + 1.0 / 256.0))
    ) * np.float32(NEAR + 1.0)
    t = np.concatenate([t_close, t_far]).astype(np.float32)
    dist = (t[1:] - t[:-1]).astype(np.float32)
    tv = t[:-1]
    return tv, dist


def _host_prep(grid, W1, b1, W2, b2, Ws, bs, Wr1, br1, Wr2, br2):
    g = np.asarray(grid) > 0.0
    mp = np.zeros((MPAD, MPAD, MPAD), dtype=np.uint8)
    t1 = g[:, :, :-1] | g[:, :, 1:]
    tx = np.zeros((128, 128, 129), dtype=bool)
    tx[:, :, 1:128] = t1
    tx[:, :, 0] = g[:, :, 0]
    tx[:, :, 128] = g[:, :, 127]
    t2 = tx[:, :-1, :] | tx[:, 1:, :]
    ty = np.zeros((128, 129, 129), dtype=bool)
    ty[:, 1:128, :] = t2
    ty[:, 0, :] = tx[:, 0, :]
    ty[:, 128, :] = tx[:, 127, :]
    t3 = ty[:-1, :, :] | ty[1:, :, :]
    tz = np.zeros((129, 129, 129), dtype=bool)
    tz[1:128, :, :] = t3
    tz[0, :, :] = ty[0, :, :]
    tz[128, :, :] = ty[127, :, :]
    mp[0:129, 0:129, 0:129] = tz

    W1 = np.asarray(W1, np.float32); b1 = np.asarray(b1, np.float32)
    W2 = np.asarray(W2, np.float32); b2 = np.asarray(b2, np.float32)
    Ws = np.asarray(Ws, np.float32); bs = np.asarray(bs, np.float32)
    Wr1 = np.asarray(Wr1, np.float32); br1 = np.asarray(br1, np.float32)
    Wr2 = np.asarray(Wr2, np.float32); br2 = np.asarray(br2, np.float32)

    W1s = W1 / 64.0
    b1p = b1 - (64.5 / 64.0) * W1.sum(axis=0)
    A1 = np.zeros((6, 128), np.float32)
    A1[0:3, 0:64] = W1s
    A1[3:6, 64:128] = W1s
    b1c = np.concatenate([b1p, b1p]).astype(np.float32)

    W2e = np.concatenate([W2, W2 @ Ws], axis=1)
    A2 = np.zeros((128, 66), np.float32)
    A2[0:64, 0:33] = W2e
    A2[64:128, 33:66] = W2e

    A3 = np.zeros((70, 128), np.float32)
    A3[0:35, 0:64] = Wr1
    A3[35:70, 64:128] = Wr1
    br1p = br1 + b2 @ Wr1[0:32, :]
    b3c = np.concatenate([br1p, br1p]).astype(np.float32)

    A4 = np.zeros((128, 64), np.float32)
    A4[0:64, 0:3] = Wr2
    A4[64:128, 3:6] = Wr2

    bsp = float(bs[0] + b2 @ Ws[:, 0])
    sigb = np.full(128, bsp, np.float32)
    br2c = np.repeat(-br2[:, None], 128, axis=1).astype(np.float32)

    tv, dist = _t_tables(NS)
    t_pad = np.concatenate([tv, tv[-1:]]).astype(np.float32)
    nd_pad = np.concatenate([-dist, np.zeros(1, np.float32)]).astype(np.float32)

    return dict(mp=mp.reshape(-1), a1=A1, b1c=b1c, a2=A2, a3=A3, b3c=b3c,
                a4=A4, sigb=sigb, br2c=br2c, tpad=t_pad, ndist=nd_pad)


# --------------------------------------------------------------------------
# Bass kernel builder
# --------------------------------------------------------------------------

def _build_bass():
    for p in _BASS_PATHS:
        if os.path.isdir(p) and p not in sys.path:
            sys.path.append(p)
    import concourse.bass as bass
    import concourse.mybir as mybir
    import concourse.tile as tile
    from contextlib import ExitStack

    f32 = mybir.dt.float32
    i32 = mybir.dt.int32
    u8 = mybir.dt.uint8
    ALU = mybir.AluOpType
    ACTF = mybir.ActivationFunctionType
    AX = mybir.AxisListType

    P = 128
    NT = 128
    NB = NRAYS_CORE // P
    CH_RAYS = 32
    NCH = NRAYS_CORE // CH_RAYS
    CCOLS = CH_RAYS * 64
    NPC = CCOLS // 512

    input_specs = [
        ("ro", [NRAYS_CORE, 3], f32),
        ("rd", [NRAYS_CORE, 3], f32),
        ("mp", [MPAD * MPAD * MPAD], u8),
        ("a1", [6, 128], f32),
        ("b1c", [128], f32),
        ("a2", [128, 66], f32),
        ("a3", [70, 128], f32),
        ("b3c", [128], f32),
        ("a4", [128, 64], f32),
        ("sigb", [128], f32),
        ("br2c", [3, 128], f32),
        ("tpad", [128], f32),
        ("ndist", [128], f32),
    ]

    nc = bass.Bass("TRN2", target_bir_lowering=False, debug=False)
    ins = {
        name: nc.dram_tensor(name, shape, dt, kind="ExternalInput").ap()
        for name, shape, dt in input_specs
    }
    out_ap = nc.dram_tensor("out", [NRAYS_CORE, 3], f32, kind="ExternalOutput").ap()

    with tile.TileContext(nc) as tc, ExitStack() as ctx:
        cpool = ctx.enter_context(tc.tile_pool(name="consts", bufs=1))
        gpool = ctx.enter_context(tc.tile_pool(name="geom", bufs=3))
        ppool = ctx.enter_context(tc.tile_pool(name="persist", bufs=1))
        fpool = ctx.enter_context(tc.tile_pool(name="fm", bufs=2))
        epool = ctx.enter_context(tc.tile_pool(name="epi", bufs=2))
        ps1p = ctx.enter_context(tc.tile_pool(name="ps1", bufs=2, space="PSUM"))
        ps2p = ctx.enter_context(tc.tile_pool(name="ps2", bufs=2, space="PSUM"))
        ps3p = ctx.enter_context(tc.tile_pool(name="ps3", bufs=2, space="PSUM"))
        ps4p = ctx.enter_context(tc.tile_pool(name="ps4", bufs=2, space="PSUM"))

        t_tile = cpool.tile([P, NT], f32, name="t_tile")
        nc.sync.dma_start(out=t_tile[:],
                          in_=ins["tpad"][None, :].to_broadcast([P, NT]))
        nd_tile = cpool.tile([P, NT], f32, name="nd_tile")
        nc.sync.dma_start(out=nd_tile[:],
                          in_=ins["ndist"][None, :].to_broadcast([P, NT]))
        a1sb = cpool.tile([6, 128], f32, name="a1sb")
        nc.sync.dma_start(out=a1sb[:], in_=ins["a1"][:])
        a2sb = cpool.tile([128, 66], f32, name="a2sb")
        nc.sync.dma_start(out=a2sb[:], in_=ins["a2"][:])
        a3sb = cpool.tile([70, 128], f32, name="a3sb")
        nc.sync.dma_start(out=a3sb[:], in_=ins["a3"][:])
        a4sb = cpool.tile([128, 64], f32, name="a4sb")
        nc.sync.dma_start(out=a4sb[:], in_=ins["a4"][:])
        b1sb = cpool.tile([128, 1], f32, name="b1sb")
        nc.sync.dma_start(out=b1sb[:], in_=ins["b1c"][:, None])
        b3sb = cpool.tile([128, 1], f32, name="b3sb")
        nc.sync.dma_start(out=b3sb[:], in_=ins["b3c"][:, None])
        sigbsb = cpool.tile([128, 1], f32, name="sigbsb")
        nc.sync.dma_start(out=sigbsb[:], in_=ins["sigb"][:, None])
        br2sb = cpool.tile([128, 3], f32, name="br2sb")
        for c in range(3):
            nc.sync.dma_start(out=br2sb[:, c : c + 1],
                              in_=ins["br2c"][c, :][:, None])
        zero512 = cpool.tile([P, 512], f32, name="zero512")
        nc.vector.memset(zero512[:], 0.0)

        Gx = ppool.tile([P, NRAYS_CORE], f32, name="gx")
        Gy = ppool.tile([P, NRAYS_CORE], f32, name="gy")
        Gz = ppool.tile([P, NRAYS_CORE], f32, name="gz")
        Gk = [Gx, Gy, Gz]
        idxt = ppool.tile([P, NRAYS_CORE], i32, name="idx")
        mask8 = ppool.tile([P, NRAYS_CORE], u8, name="mask8")
        sig_rm = ppool.tile([P, NRAYS_CORE], f32, name="sigrm")
        rgb_rm = [ppool.tile([P, NRAYS_CORE], f32, name=f"rgbrm{c}")
                  for c in range(3)]

        mp2d = ins["mp"][:, None]

        # ---------------- geometry + occupancy gather ----------------
        for b in range(NB):
            bs_ = slice(b * NT, (b + 1) * NT)
            rod = gpool.tile([P, 3], f32, tag="rod", name="rod")
            nc.sync.dma_start(out=rod[:], in_=ins["ro"][b * P : (b + 1) * P, :])
            rdd = gpool.tile([P, 3], f32, tag="rdd", name="rdd")
            nc.sync.dma_start(out=rdd[:], in_=ins["rd"][b * P : (b + 1) * P, :])

            S = [gpool.tile([P, NT], f32, tag=f"s{k}", name=f"s{k}")
                 for k in range(3)]
            Ab = [gpool.tile([P, NT], f32, tag=f"ab{k}", name=f"ab{k}")
                  for k in range(3)]
            for k in range(3):
                nc.vector.tensor_scalar(
                    out=S[k][:], in0=t_tile[:],
                    scalar1=rdd[:, k : k + 1], scalar2=rod[:, k : k + 1],
                    op0=ALU.mult, op1=ALU.add)
                nc.vector.scalar_tensor_tensor(
                    out=Ab[k][:], in0=S[k][:], scalar=-1.0, in1=S[k][:],
                    op0=ALU.mult, op1=ALU.max)
            nrm = gpool.tile([P, NT], f32, tag="nrm", name="nrm")
            nc.vector.tensor_tensor(out=nrm[:], in0=Ab[0][:], in1=Ab[1][:],
                                    op=ALU.max)
            nc.vector.tensor_tensor(out=nrm[:], in0=nrm[:], in1=Ab[2][:],
                                    op=ALU.max)
            nc.vector.tensor_scalar(out=nrm[:], in0=nrm[:], scalar1=1.0,
                                    scalar2=None, op0=ALU.max)
            rcp = gpool.tile([P, NT], f32, tag="rcp", name="rcp")
            nc.vector.reciprocal(out=rcp[:], in_=nrm[:])
            gm = gpool.tile([P, NT], f32, tag="gm", name="gm")
            nc.vector.tensor_scalar(out=gm[:], in0=rcp[:], scalar1=-32.0,
                                    scalar2=64.0, op0=ALU.mult, op1=ALU.add)
            nc.vector.tensor_tensor(out=gm[:], in0=gm[:], in1=rcp[:],
                                    op=ALU.mult)

            I = [gpool.tile([P, NT], f32, tag=f"i{k}", name=f"i{k}")
                 for k in range(3)]
            for k in range(3):
                gsl = Gk[k][:, bs_]
                nc.vector.tensor_tensor(out=gsl, in0=S[k][:], in1=gm[:],
                                        op=ALU.mult)
                nc.vector.tensor_scalar(out=gsl, in0=gsl, scalar1=64.5,
                                        scalar2=None, op0=ALU.add)
                frk = Ab[k]
                nc.vector.tensor_scalar(out=frk[:], in0=gsl, scalar1=1.0,
                                        scalar2=None, op0=ALU.mod)
                nc.vector.tensor_tensor(out=I[k][:], in0=gsl, in1=frk[:],
                                        op=ALU.subtract)
            idxf = gpool.tile([P, NT], f32, tag="idxf", name="idxf")
            nc.vector.scalar_tensor_tensor(
                out=idxf[:], in0=I[2][:], scalar=float(MPAD), in1=I[1][:],
                op0=ALU.mult, op1=ALU.add)
            nc.vector.scalar_tensor_tensor(
                out=idxf[:], in0=idxf[:], scalar=float(MPAD), in1=I[0][:],
                op0=ALU.mult, op1=ALU.add)
            nc.vector.tensor_copy(out=idxt[:, bs_], in_=idxf[:])
            nc.gpsimd.indirect_dma_start(
                out=mask8[:, bs_],
                out_offset=None,
                in_=mp2d,
                in_offset=bass.IndirectOffsetOnAxis(ap=idxt[:, bs_], axis=0),
            )

        # ---------------- MLP over 2-sample-packed chunks ----------------
        for ci in range(NCH):
            b = ci // (P // CH_RAYS)
            q = ci % (P // CH_RAYS)
            r0 = q * CH_RAYS
            gr0 = b * P + r0
            bs0 = b * NT

            x6 = fpool.tile([6, CCOLS], f32, tag="x6", name="x6")
            for j in range(6):
                k = j % 3
                th = (j // 3) * 64
                src = Gk[k][r0 : r0 + CH_RAYS, bs0 + th : bs0 + th + 64]
                nc.sync.dma_start(out=x6[j : j + 1, :], in_=src)

            h1 = fpool.tile([128, CCOLS], f32, tag="h1", name="h1")
            for n in range(NPC):
                psl = slice(512 * n, 512 * (n + 1))
                ps1 = ps1p.tile([128, 512], f32, tag="ps1", name="ps1")
                nc.tensor.matmul(ps1[:], lhsT=a1sb[:], rhs=x6[:, psl],
                                 start=True, stop=True)
                nc.scalar.activation(out=h1[:, psl], in_=ps1[:],
                                     func=ACTF.Relu, bias=b1sb[:])

            st66 = fpool.tile([66, CCOLS], f32, tag="st66", name="st66")
            for n in range(NPC):
                psl = slice(512 * n, 512 * (n + 1))
                ps2 = ps2p.tile([66, 512], f32, tag="ps2", name="ps2")
                nc.tensor.matmul(ps2[:], lhsT=a2sb[:], rhs=h1[:, psl],
                                 start=True, stop=True)
                nc.gpsimd.tensor_copy(out=st66[:, psl], in_=ps2[:])

            xr = fpool.tile([70, CCOLS], f32, tag="xr", name="xr")
            nc.sync.dma_start(out=xr[0:32, :], in_=st66[0:32, :])
            nc.sync.dma_start(out=xr[35:67, :], in_=st66[33:65, :])
            for d in range(3):
                dsrc = ins["rd"][gr0 : gr0 + CH_RAYS, d][:, None].to_broadcast(
                    [CH_RAYS, 64])
                nc.sync.dma_start(out=xr[32 + d : 33 + d, :], in_=dsrc)
                nc.sync.dma_start(out=xr[67 + d : 68 + d, :], in_=dsrc)

            nc.sync.dma_start(
                out=sig_rm[r0 : r0 + CH_RAYS, bs0 : bs0 + 64],
                in_=st66[32:33, :].rearrange("p (a t) -> p a t", a=CH_RAYS))
            nc.sync.dma_start(
                out=sig_rm[r0 : r0 + CH_RAYS, bs0 + 64 : bs0 + 128],
                in_=st66[65:66, :].rearrange("p (a t) -> p a t", a=CH_RAYS))

            hr = fpool.tile([128, CCOLS], f32, tag="hr", name="hr")
            for n in range(NPC):
                psl = slice(512 * n, 512 * (n + 1))
                ps3 = ps3p.tile([128, 512], f32, tag="ps3", name="ps3")
                nc.tensor.matmul(ps3[:], lhsT=a3sb[:], rhs=xr[:, psl],
                                 start=True, stop=True)
                if n % 2 == 0:
                    nc.scalar.activation(out=hr[:, psl], in_=ps3[:],
                                         func=ACTF.Relu, bias=b3sb[:])
                else:
                    nc.vector.scalar_tensor_tensor(
                        out=hr[:, psl], in0=ps3[:], scalar=b3sb[:],
                        in1=zero512[:], op0=ALU.add, op1=ALU.max)

            st24s = []
            for bank in range(NPC // 2):
                ps4 = ps4p.tile([128, 512], f32, tag="ps4", name="ps4")
                for m in range(2):
                    n = 2 * bank + m
                    psl = slice(512 * n, 512 * (n + 1))
                    nc.tensor.matmul(ps4[64 * m : 64 * m + 64, :],
                                     lhsT=a4sb[:], rhs=hr[:, psl],
                                     start=True, stop=True)
                st24 = fpool.tile([128, 512], f32, tag=f"st24_{bank}",
                                  name=f"st24_{bank}")
                nc.gpsimd.tensor_copy(out=st24[:], in_=ps4[:])
                st24s.append(st24)

            for c in range(3):
                for half in range(2):
                    for n in range(NPC):
                        row = 64 * (n % 2) + 3 * half + c
                        src = st24s[n // 2][row : row + 1, :].rearrange(
                            "p (a t) -> p a t", a=8)
                        dst = rgb_rm[c][
                            r0 + 8 * n : r0 + 8 * n + 8,
                            bs0 + 64 * half : bs0 + 64 * half + 64,
                        ]
                        nc.sync.dma_start(out=dst, in_=src)

        # ---------------- ray-major epilogue ----------------
        for b in range(NB):
            bs_ = slice(b * NT, (b + 1) * NT)
            mf = epool.tile([P, NT], f32, tag="mf", name="mf")
            nc.vector.tensor_copy(out=mf[:], in_=mask8[:, bs_])
            sg = epool.tile([P, NT], f32, tag="sg", name="sg")
            nc.vector.tensor_scalar(
                out=sg[:], in0=sig_rm[:, bs_], scalar1=sigbsb[:],
                scalar2=80.0, op0=ALU.add, op1=ALU.min)
            nc.scalar.activation(out=sg[:], in_=sg[:], func=ACTF.Exp)
            nc.vector.tensor_scalar(out=sg[:], in0=sg[:], scalar1=1.0,
                                    scalar2=None, op0=ALU.add)
            nc.scalar.activation(out=sg[:], in_=sg[:], func=ACTF.Ln)
            nc.vector.tensor_tensor(out=sg[:], in0=sg[:], in1=mf[:],
                                    op=ALU.mult)
            al = epool.tile([P, NT], f32, tag="al", name="al")
            nc.vector.tensor_tensor(out=al[:], in0=sg[:], in1=nd_tile[:],
                                    op=ALU.mult)
            E = epool.tile([P, NT], f32, tag="E", name="E")
            nc.scalar.activation(out=E[:], in_=al[:], func=ACTF.Exp)
            Pt = epool.tile([P, NT], f32, tag="Pt", name="Pt")
            nc.vector.tensor_tensor_scan(
                out=Pt[:], data0=E[:], data1=E[:], initial=1.0,
                op0=ALU.mult, op1=ALU.bypass)
            w = epool.tile([P, NT], f32, tag="w", name="w")
            nc.vector.tensor_scalar(
                out=w[:, 0:1], in0=Pt[:, 0:1], scalar1=-1.0, scalar2=1.0,
                op0=ALU.mult, op1=ALU.add)
            nc.vector.tensor_tensor(out=w[:, 1:NT], in0=Pt[:, 0 : NT - 1],
                                    in1=Pt[:, 1:NT], op=ALU.subtract)
            wt = epool.tile([P, NT], f32, tag="wt", name="wt")
            nc.vector.scalar_tensor_tensor(
                out=wt[:], in0=w[:], scalar=EARLY_TERM, in1=w[:],
                op0=ALU.is_gt, op1=ALU.mult)
            nc.vector.tensor_tensor(out=wt[:], in0=wt[:], in1=mf[:],
                                    op=ALU.mult)
            nc.vector.memset(wt[:, NT - 1 : NT], 0.0)

            outb = epool.tile([P, 3], f32, tag="outb", name="outb")
            for c in range(3):
                rs = epool.tile([P, NT], f32, tag=f"rs{c}", name=f"rs{c}")
                nc.scalar.activation(out=rs[:], in_=rgb_rm[c][:, bs_],
                                     func=ACTF.Exp, scale=-1.0,
                                     bias=br2sb[:, c : c + 1])
                nc.vector.tensor_scalar(out=rs[:], in0=rs[:], scalar1=1.0,
                                        scalar2=None, op0=ALU.add)
                nc.vector.reciprocal(out=rs[:], in_=rs[:])
                nc.vector.tensor_tensor(out=rs[:], in0=rs[:], in1=wt[:],
                                        op=ALU.mult)
                nc.vector.tensor_reduce(out=outb[:, c : c + 1], in_=rs[:],
                                        axis=AX.X, op=ALU.add)
            nc.sync.dma_start(out=out_ap[b * P : (b + 1) * P, :], in_=outb[:])

    return nc


# --------------------------------------------------------------------------
# numpy fallback (exact reference math)
# --------------------------------------------------------------------------

def _render_numpy(ro, rd, grid, W1, b1, W2, b2, Ws, bs, Wr1, br1, Wr2, br2,
                  tv, dist):
    samples = ro[:, None, :] + rd[:, None, :] * tv[None, :, None]
    norm = np.max(np.abs(samples), axis=-1, keepdims=True)
    ns = np.maximum(norm, 1.0)
    sc = np.where(norm <= 1.0, samples, (2.0 - 1.0 / ns) * samples / ns) / 2.0

    D = H = W = GS
    x = ((sc[..., 0] + 1.0) * W - 1.0) * 0.5
    y = ((sc[..., 1] + 1.0) * H - 1.0) * 0.5
    z = ((sc[..., 2] + 1.0) * D - 1.0) * 0.5
    x0 = np.floor(x); y0 = np.floor(y); z0 = np.floor(z)
    fx = (x - x0).astype(np.float32)
    fy = (y - y0).astype(np.float32)
    fz = (z - z0).astype(np.float32)
    x0 = x0.astype(np.int32); y0 = y0.astype(np.int32); z0 = z0.astype(np.int32)

    def corner(zi, yi, xi):
        valid = (zi >= 0) & (zi < D) & (yi >= 0) & (yi < H) & (xi >= 0) & (xi < W)
        v = grid[np.clip(zi, 0, D - 1), np.clip(yi, 0, H - 1), np.clip(xi, 0, W - 1)]
        return v * valid.astype(grid.dtype)

    occ = (
        corner(z0, y0, x0) * (1 - fz) * (1 - fy) * (1 - fx)
        + corner(z0, y0, x0 + 1) * (1 - fz) * (1 - fy) * fx
        + corner(z0, y0 + 1, x0) * (1 - fz) * fy * (1 - fx)
        + corner(z0, y0 + 1, x0 + 1) * (1 - fz) * fy * fx
        + corner(z0 + 1, y0, x0) * fz * (1 - fy) * (1 - fx)
        + corner(z0 + 1, y0, x0 + 1) * fz * (1 - fy) * fx
        + corner(z0 + 1, y0 + 1, x0) * fz * fy * (1 - fx)
        + corner(z0 + 1, y0 + 1, x0 + 1) * fz * fy * fx
    )
    mask = occ > 0.0
    maskf = mask.astype(np.float32)

    relu = lambda v: np.maximum(v, 0.0)
    feat = relu(sc @ W1 + b1) @ W2 + b2
    feat = feat * maskf[..., None]
    s_in = (feat @ Ws + bs)[..., 0]
    sigma = (np.logaddexp(0.0, s_in) * maskf).astype(np.float32)

    alpha_log = -sigma * dist[None, :]
    trans = np.exp(np.cumsum(alpha_log, axis=1))
    n = ro.shape[0]
    trans = np.concatenate([np.ones((n, 1), np.float32), trans[:, :-1]], axis=1)
    alpha = 1.0 - np.exp(alpha_log)
    weights = (trans * alpha).astype(np.float32)

    mask2 = mask & (weights > EARLY_TERM)
    dirs = np.broadcast_to(rd[:, None, :], samples.shape)
    h = relu(np.concatenate([feat, dirs], axis=-1) @ Wr1 + br1)
    sig = 1.0 / (1.0 + np.exp(-(h @ Wr2 + br2)))
    rgb = sig * weights[..., None]
    rgb = rgb * mask2[..., None].astype(np.float32)
    return rgb.sum(axis=1).astype(np.float32)


# --------------------------------------------------------------------------
# entry point
# --------------------------------------------------------------------------

def kernel(rays_o, rays_d, grid, W1, b1, W2, b2, Ws, bs, Wr1, br1, Wr2, br2,
           n_samples=NS):
    rays_o = np.ascontiguousarray(np.asarray(rays_o, dtype=np.float32))
    rays_d = np.ascontiguousarray(np.asarray(rays_d, dtype=np.float32))
    grid = np.asarray(grid, dtype=np.float32)
    consts_raw = [np.asarray(a, dtype=np.float32)
                  for a in (W1, b1, W2, b2, Ws, bs, Wr1, br1, Wr2, br2)]
    ns_val = int(np.asarray(n_samples))

    result = {}

    def _device_path():
        assert ns_val == NS and rays_o.shape == (N_RAYS, 3)
        assert grid.shape == (GS, GS, GS)
        cd = _host_prep(grid, *consts_raw)

        if "nc" not in _cache:
            _cache["nc"] = _build_bass()
        nc = _cache["nc"]

        from concourse.bass_utils import run_bass_kernel_spmd

        in_maps = []
        for c in range(N_CORES):
            sl = slice(NRAYS_CORE * c, NRAYS_CORE * (c + 1))
            m = {"ro": rays_o[sl], "rd": rays_d[sl]}
            m.update({k: cd[k] for k in
                      ("mp", "a1", "b1c", "a2", "a3", "b3c", "a4", "sigb",
                       "br2c", "tpad", "ndist")})
            in_maps.append(m)

        res = run_bass_kernel_spmd(nc, in_maps, core_ids=list(range(N_CORES)))
        out = np.concatenate([r["out"] for r in res.results], axis=0)
        assert out.shape == (N_RAYS, 3) and np.isfinite(out).all()
        result["out"] = out.astype(np.float32)

    budget_s = float(os.environ.get("KERNEL_DEVICE_TIMEOUT_S", "900"))
    th = threading.Thread(target=_device_path, daemon=True)
    th.start()
    th.join(timeout=budget_s)
    if "out" in result:
        return result["out"]

    tv, dist = _t_tables(ns_val)
    return _render_numpy(rays_o, rays_d, grid, *consts_raw, tv, dist)
